# revision 8
# baseline (speedup 1.0000x reference)
"""LocalVarianceMap Trainium2 kernel.

reference:
  lum  = mean over channel of x            (B,1,H,W)
  mean = 7x7 'same' box mean of lum ; sqm = same of lum^2
  out  = sqm - mean^2

Full input x: (16, 3, 1024, 1024) fp32. Data-parallel over batch:
8 NeuronCores x 2 images each.

Per-core pipeline per 128-row tile (partition=h, free=w):
  DMA in 3 channel blocks               (SP HWDGE ring)
  lum = x0+x1+x2                        (GPSIMD, 2 tensor adds)
  sq  = lum^2                           (ACT Square)
  h1/h2 = sliding 7-sum along w         (DVE tensor_tensor_scan x2)
  S1 = band^T @ h1   (PE, float32r)     vertical 7-sum, mean path
  S2 = band^T @ h2   (PE, float32)      vertical 7-sum, sq path (exact)
  m2 = Square(S1/147) (ACT -> float32r)
  S2 += (-441 I) @ m2 (PE, float32r)    fold -441*mean^2 into PSUM
  out = Copy(S2 * 1/441) (ACT)
  DMA out in 2 halves                   (ACT HWDGE ring)
Tiles overlap by 6 input rows so the vertical halo lives in-tile.
"""

import sys

if "/opt/trn_rl_repo" not in sys.path:
    sys.path.insert(0, "/opt/trn_rl_repo")

import numpy as np
from contextlib import ExitStack

import concourse.bass as bass
import concourse.bacc as bacc
import concourse.tile as tile
from concourse import mybir

H = 1024
W = 1024
C = 3
PER_CORE_B = 2
N_CORES = 8
K7 = 7
PADL, PADR = 7, 3
LW = PADL + W + PADR      # padded lum/sq width (1034)
SCAN_N = W + 3            # h[:, j+3] = centered 7-sum at col j

MEAN_F32R = True          # mean-path vertical matmul + m2 fold in float32r


def _tiles():
    specs = []
    specs.append(dict(r0=0, nr=128, K=128, M=125, out_r0=0, w=0))
    for t in range(1, 8):
        specs.append(dict(r0=122 * t, nr=128, K=128, M=122, out_r0=122 * t + 3, w=1))
    specs.append(dict(r0=976, nr=48, K=48, M=45, out_r0=979, w=2))
    assert specs[-1]["out_r0"] + specs[-1]["M"] == H
    return specs


def band_weights() -> np.ndarray:
    """Four [128,128] blocks: W0 | Wmid | Wlast | -441*I."""
    wb = np.zeros((128, 4 * 128), np.float32)
    for m in range(125):
        for k in range(max(m - 3, 0), m + 4):
            wb[k, m] = 1.0
    for m in range(122):
        for k in range(m, m + 7):
            wb[k, 128 + m] = 1.0
    for m in range(45):
        for k in range(m, min(m + 7, 48)):
            wb[k, 256 + m] = 1.0
    for m in range(128):
        wb[m, 384 + m] = -441.0
    return wb


def build_nc(finalize: bool = True) -> bass.Bass:
    nc = bacc.Bacc("TRN2", target_bir_lowering=False)
    f32 = mybir.dt.float32
    f32r = mybir.dt.float32r
    wdt = f32r if MEAN_F32R else f32

    x = nc.dram_tensor("x", [PER_CORE_B, C, H, W], f32, kind="ExternalInput")
    wbt = nc.dram_tensor("wb", [128, 4 * 128], wdt, kind="ExternalInput")
    y = nc.dram_tensor("y", [PER_CORE_B, 1, H, W], f32, kind="ExternalOutput")

    inv147 = float(np.float32(1.0) / np.float32(147.0))
    inv441 = float(np.float32(1.0) / np.float32(441.0))

    with tile.TileContext(nc) as tc, ExitStack() as ctx:
        cpool = ctx.enter_context(tc.tile_pool(name="const", bufs=1))
        xpool = ctx.enter_context(tc.tile_pool(name="xin", bufs=4))
        lpool = ctx.enter_context(tc.tile_pool(name="lum", bufs=3))
        spool = ctx.enter_context(tc.tile_pool(name="sq", bufs=3))
        hpool = ctx.enter_context(tc.tile_pool(name="hsum", bufs=3))
        mpool = ctx.enter_context(tc.tile_pool(name="m2", bufs=3))
        vpool = ctx.enter_context(tc.tile_pool(name="vout", bufs=4))
        ppool = ctx.enter_context(tc.tile_pool(name="psum", bufs=2, space="PSUM"))

        WB = cpool.tile([128, 4 * 128], wdt)
        nc.sync.dma_start(out=WB[:], in_=wbt[:, :])
        WBf = WB[:].bitcast(f32) if MEAN_F32R else WB[:]

        for b in range(PER_CORE_B):
            for sp in _tiles():
                r0, nr, K, M, out_r0, wsel = (
                    sp["r0"], sp["nr"], sp["K"], sp["M"], sp["out_r0"], sp["w"],
                )
                X = xpool.tile([128, C * W], f32, tag="X")
                for c in range(C):
                    nc.sync.dma_start(
                        out=X[0:nr, c * W : (c + 1) * W],
                        in_=x[b, c, r0 : r0 + nr, :],
                    )

                lum = lpool.tile([128, LW], f32, tag="lum")
                nc.gpsimd.memset(lum[0:nr, 0:PADL], 0.0)
                nc.gpsimd.memset(lum[0:nr, PADL + W : LW], 0.0)
                nc.gpsimd.tensor_add(
                    lum[0:nr, PADL : PADL + W], X[0:nr, 0:W], X[0:nr, W : 2 * W]
                )
                nc.gpsimd.tensor_add(
                    lum[0:nr, PADL : PADL + W],
                    lum[0:nr, PADL : PADL + W],
                    X[0:nr, 2 * W : 3 * W],
                )

                sq = spool.tile([128, LW], f32, tag="sq")
                nc.gpsimd.memset(sq[0:nr, 0:PADL], 0.0)
                nc.gpsimd.memset(sq[0:nr, PADL + W : LW], 0.0)
                nc.scalar.activation(
                    sq[0:nr, PADL : PADL + W],
                    lum[0:nr, PADL : PADL + W],
                    mybir.ActivationFunctionType.Square,
                )

                h1 = hpool.tile([128, SCAN_N], f32r if MEAN_F32R else f32, tag="h1")
                h2 = hpool.tile([128, SCAN_N], f32, tag="h2")
                nc.vector.tensor_tensor_scan(
                    out=h1[0:nr, :],
                    data0=lum[0:nr, PADL : PADL + SCAN_N],
                    data1=lum[0:nr, 0:SCAN_N],
                    initial=0.0,
                    op0=mybir.AluOpType.add,
                    op1=mybir.AluOpType.subtract,
                )
                nc.vector.tensor_tensor_scan(
                    out=h2[0:nr, :],
                    data0=sq[0:nr, PADL : PADL + SCAN_N],
                    data1=sq[0:nr, 0:SCAN_N],
                    initial=0.0,
                    op0=mybir.AluOpType.add,
                    op1=mybir.AluOpType.subtract,
                )

                S = ppool.tile([128, 2 * W], f32, tag="S")
                # mean path (float32r full-rate)
                for cnk in range(2):
                    nc.tensor.matmul(
                        S[0:M, 512 * cnk : 512 * (cnk + 1)],
                        WB[0:K, 128 * wsel : 128 * wsel + M],
                        h1[0:K, 3 + 512 * cnk : 3 + 512 * (cnk + 1)],
                        start=True,
                        stop=True,
                    )
                # sq path (exact fp32)
                for cnk in range(2):
                    nc.tensor.matmul(
                        S[0:M, W + 512 * cnk : W + 512 * (cnk + 1)],
                        WBf[0:K, 128 * wsel : 128 * wsel + M],
                        h2[0:K, 3 + 512 * cnk : 3 + 512 * (cnk + 1)],
                        start=True,
                        stop=False,
                    )

                m2 = mpool.tile([128, W], f32r if MEAN_F32R else f32, tag="m2")
                nc.scalar.activation(
                    m2[0:M, :],
                    S[0:M, 0:W],
                    mybir.ActivationFunctionType.Square,
                    scale=inv147,
                )

                # fold -441*mean^2 into the sq-path PSUM chunks
                for cnk in range(2):
                    nc.tensor.matmul(
                        S[0:M, W + 512 * cnk : W + 512 * (cnk + 1)],
                        WB[0:M, 384 : 384 + M],
                        m2[0:M, 512 * cnk : 512 * (cnk + 1)],
                        start=False,
                        stop=True,
                    )

                V = vpool.tile([128, W], f32, tag="V")
                nc.scalar.activation(
                    V[0:M, :],
                    S[0:M, W : 2 * W],
                    mybir.ActivationFunctionType.Copy,
                    scale=inv441,
                )

                # out in 2 halves on the ACT HWDGE ring (DMA engine balance)
                h_half = (M + 1) // 2
                nc.scalar.dma_start(
                    out=y[b, 0, out_r0 : out_r0 + h_half, :], in_=V[0:h_half, :]
                )
                nc.scalar.dma_start(
                    out=y[b, 0, out_r0 + h_half : out_r0 + M, :], in_=V[h_half:M, :]
                )

    if finalize:
        nc.finalize()
    return nc


def kernel(x, kernel_size):
    assert int(kernel_size) == K7
    x = np.ascontiguousarray(np.asarray(x, dtype=np.float32))
    B = x.shape[0]
    assert x.shape == (B, C, H, W) and B == PER_CORE_B * N_CORES

    from concourse.bass_utils import run_bass_kernel_spmd

    nc = build_nc()
    wb = band_weights()
    in_maps = [
        {"x": x[i * PER_CORE_B : (i + 1) * PER_CORE_B], "wb": wb}
        for i in range(N_CORES)
    ]
    res = run_bass_kernel_spmd(nc, in_maps, list(range(N_CORES)))
    y = np.concatenate([res.results[i]["y"] for i in range(N_CORES)], axis=0)
    return y


# revision 9
# speedup vs baseline: 1.4762x; 1.4762x over previous
"""LocalVarianceMap Trainium2 kernel.

reference:
  lum  = mean over channel of x            (B,1,H,W)
  mean = 7x7 'same' box mean of lum ; sqm = same of lum^2
  out  = sqm - mean^2

Full input x: (16, 3, 1024, 1024) fp32. Data-parallel over batch:
8 NeuronCores x 2 images each.

Per-core pipeline per 128-row tile (partition=h, free=w):
  lum = x0+x1+x2 built by DMA: plain HWDGE load of channel 0 into the
    padded lum tile, then two SWDGE accumulate-DMAs (channels 1, 2).
  sq  = lum^2                            (ACT Square)
  h1/h2 = sliding 7-sum along w          (DVE tensor_tensor_scan x2)
  S1 = band^T @ h1      (PE float32r)    vertical 7-sum, mean path
  m2 = Square(S1/147)   (ACT -> float32r)
  S2 chunk: (-441 I) @ m2 written first (PE float32r, start=True),
            then band^T @ h2 accumulates in exact fp32 (PE float32)
  out = Copy(S2 * 1/441) (ACT)
  DMA out via SWDGE (spreads across all 16 SDMA engines)
Tiles overlap by 6 input rows so the vertical halo lives in-tile.
"""

import sys

if "/opt/trn_rl_repo" not in sys.path:
    sys.path.insert(0, "/opt/trn_rl_repo")

import numpy as np
from contextlib import ExitStack

import concourse.bass as bass
import concourse.bacc as bacc
import concourse.tile as tile
from concourse import mybir

H = 1024
W = 1024
C = 3
PER_CORE_B = 2
N_CORES = 8
K7 = 7
PADL, PADR = 7, 3
LW = PADL + W + PADR      # padded lum/sq width (1034)
SCAN_N = W + 3            # h[:, j+3] = centered 7-sum at col j


def _tiles():
    specs = []
    specs.append(dict(r0=0, nr=128, K=128, M=125, out_r0=0, w=0))
    for t in range(1, 8):
        specs.append(dict(r0=122 * t, nr=128, K=128, M=122, out_r0=122 * t + 3, w=1))
    specs.append(dict(r0=976, nr=48, K=48, M=45, out_r0=979, w=2))
    assert specs[-1]["out_r0"] + specs[-1]["M"] == H
    return specs


def band_weights() -> np.ndarray:
    """Four [128,128] blocks: W0 | Wmid | Wlast | -441*I."""
    wb = np.zeros((128, 4 * 128), np.float32)
    for m in range(125):
        for k in range(max(m - 3, 0), m + 4):
            wb[k, m] = 1.0
    for m in range(122):
        for k in range(m, m + 7):
            wb[k, 128 + m] = 1.0
    for m in range(45):
        for k in range(m, min(m + 7, 48)):
            wb[k, 256 + m] = 1.0
    for m in range(128):
        wb[m, 384 + m] = -441.0
    return wb


def build_nc(finalize: bool = True) -> bass.Bass:
    nc = bacc.Bacc("TRN2", target_bir_lowering=False)
    f32 = mybir.dt.float32
    f32r = mybir.dt.float32r

    x = nc.dram_tensor("x", [PER_CORE_B, C, H, W], f32, kind="ExternalInput")
    wbt = nc.dram_tensor("wb", [128, 4 * 128], f32r, kind="ExternalInput")
    y = nc.dram_tensor("y", [PER_CORE_B, 1, H, W], f32, kind="ExternalOutput")

    inv147 = float(np.float32(1.0) / np.float32(147.0))
    inv441 = float(np.float32(1.0) / np.float32(441.0))

    with tile.TileContext(nc) as tc, ExitStack() as ctx:
        cpool = ctx.enter_context(tc.tile_pool(name="const", bufs=1))
        lpool = ctx.enter_context(tc.tile_pool(name="lum", bufs=3))
        spool = ctx.enter_context(tc.tile_pool(name="sq", bufs=3))
        hpool = ctx.enter_context(tc.tile_pool(name="hsum", bufs=3))
        mpool = ctx.enter_context(tc.tile_pool(name="m2", bufs=3))
        vpool = ctx.enter_context(tc.tile_pool(name="vout", bufs=3))
        ppool = ctx.enter_context(tc.tile_pool(name="psum", bufs=2, space="PSUM"))

        WB = cpool.tile([128, 4 * 128], f32r)
        nc.sync.dma_start(out=WB[:], in_=wbt[:, :])
        WBf = WB[:].bitcast(f32)

        for b in range(PER_CORE_B):
            for sp in _tiles():
                r0, nr, K, M, out_r0, wsel = (
                    sp["r0"], sp["nr"], sp["K"], sp["M"], sp["out_r0"], sp["w"],
                )
                lum = lpool.tile([128, LW], f32, tag="lum")
                nc.gpsimd.memset(lum[0:nr, 0:PADL], 0.0)
                nc.gpsimd.memset(lum[0:nr, PADL + W : LW], 0.0)
                # lum = x0 + x1 + x2 built by the DMA path
                nc.sync.dma_start(
                    out=lum[0:nr, PADL : PADL + W], in_=x[b, 0, r0 : r0 + nr, :]
                )
                for c in (1, 2):
                    nc.gpsimd.dma_start(
                        out=lum[0:nr, PADL : PADL + W],
                        in_=x[b, c, r0 : r0 + nr, :],
                        accum_op=mybir.AluOpType.add,
                    )

                sq = spool.tile([128, LW], f32, tag="sq")
                nc.gpsimd.memset(sq[0:nr, 0:PADL], 0.0)
                nc.gpsimd.memset(sq[0:nr, PADL + W : LW], 0.0)
                nc.scalar.activation(
                    sq[0:nr, PADL : PADL + W],
                    lum[0:nr, PADL : PADL + W],
                    mybir.ActivationFunctionType.Square,
                )

                h1 = hpool.tile([128, SCAN_N], f32r, tag="h1")
                h2 = hpool.tile([128, SCAN_N], f32, tag="h2")
                nc.vector.tensor_tensor_scan(
                    out=h1[0:nr, :],
                    data0=lum[0:nr, PADL : PADL + SCAN_N],
                    data1=lum[0:nr, 0:SCAN_N],
                    initial=0.0,
                    op0=mybir.AluOpType.add,
                    op1=mybir.AluOpType.subtract,
                )
                nc.vector.tensor_tensor_scan(
                    out=h2[0:nr, :],
                    data0=sq[0:nr, PADL : PADL + SCAN_N],
                    data1=sq[0:nr, 0:SCAN_N],
                    initial=0.0,
                    op0=mybir.AluOpType.add,
                    op1=mybir.AluOpType.subtract,
                )

                S = ppool.tile([128, 2 * W], f32, tag="S")
                # mean path (float32r full-rate)
                for cnk in range(2):
                    nc.tensor.matmul(
                        S[0:M, 512 * cnk : 512 * (cnk + 1)],
                        WB[0:K, 128 * wsel : 128 * wsel + M],
                        h1[0:K, 3 + 512 * cnk : 3 + 512 * (cnk + 1)],
                        start=True,
                        stop=True,
                    )

                m2 = mpool.tile([128, W], f32r, tag="m2")
                nc.scalar.activation(
                    m2[0:M, :],
                    S[0:M, 0:W],
                    mybir.ActivationFunctionType.Square,
                    scale=inv147,
                )

                # sq-path PSUM chunks: write -441*m2 first (small values, f32r
                # write rounding is harmless), then accumulate the exact fp32
                # band sum of h2 on top in fp32 PSUM.
                for cnk in range(2):
                    nc.tensor.matmul(
                        S[0:M, W + 512 * cnk : W + 512 * (cnk + 1)],
                        WB[0:M, 384 : 384 + M],
                        m2[0:M, 512 * cnk : 512 * (cnk + 1)],
                        start=True,
                        stop=False,
                    )
                    nc.tensor.matmul(
                        S[0:M, W + 512 * cnk : W + 512 * (cnk + 1)],
                        WBf[0:K, 128 * wsel : 128 * wsel + M],
                        h2[0:K, 3 + 512 * cnk : 3 + 512 * (cnk + 1)],
                        start=False,
                        stop=True,
                    )

                V = vpool.tile([128, W], f32, tag="V")
                nc.scalar.activation(
                    V[0:M, :],
                    S[0:M, W : 2 * W],
                    mybir.ActivationFunctionType.Copy,
                    scale=inv441,
                )

                # SWDGE out: spreads across all 16 SDMA engines
                nc.gpsimd.dma_start(
                    out=y[b, 0, out_r0 : out_r0 + M, :], in_=V[0:M, :]
                )

    if finalize:
        nc.finalize()
    return nc


def kernel(x, kernel_size):
    assert int(kernel_size) == K7
    x = np.ascontiguousarray(np.asarray(x, dtype=np.float32))
    B = x.shape[0]
    assert x.shape == (B, C, H, W) and B == PER_CORE_B * N_CORES

    from concourse.bass_utils import run_bass_kernel_spmd

    nc = build_nc()
    wb = band_weights()
    in_maps = [
        {"x": x[i * PER_CORE_B : (i + 1) * PER_CORE_B], "wb": wb}
        for i in range(N_CORES)
    ]
    res = run_bass_kernel_spmd(nc, in_maps, list(range(N_CORES)))
    y = np.concatenate([res.results[i]["y"] for i in range(N_CORES)], axis=0)
    return y


# revision 12
# speedup vs baseline: 1.7626x; 1.1940x over previous
"""LocalVarianceMap Trainium2 kernel.

reference:
  lum  = mean over channel of x            (B,1,H,W)
  mean = 7x7 'same' box mean of lum ; sqm = same of lum^2
  out  = sqm - mean^2

Full input x: (16, 3, 1024, 1024) fp32. Data-parallel over batch:
8 NeuronCores x 2 images each.

Per-core pipeline per 128-row tile (partition=h, free=w):
  lum = x0+x1+x2 built by DMA: plain HWDGE load of channel 0 into the
    padded lum tile, then two SWDGE accumulate-DMAs (channels 1, 2).
  sq  = lum^2                            (ACT Square)
  h1/h2 = sliding 7-sum along w          (DVE tensor_tensor_scan x2)
  S1 = band^T @ h1      (PE float32r)    vertical 7-sum, mean path
  m2 = Square(S1/147)   (ACT -> float32r)
  S2 chunk: (-441 I) @ m2 written first (PE float32r, start=True),
            then band^T @ h2 accumulates in exact fp32 (PE float32)
  out = Copy(S2 * 1/441) (ACT)
  DMA out via SWDGE (spreads across all 16 SDMA engines)
Tiles overlap by 6 input rows so the vertical halo lives in-tile.
"""

import sys

if "/opt/trn_rl_repo" not in sys.path:
    sys.path.insert(0, "/opt/trn_rl_repo")

import numpy as np
from contextlib import ExitStack

import concourse.bass as bass
import concourse.bacc as bacc
import concourse.tile as tile
from concourse import mybir

H = 1024
W = 1024
C = 3
PER_CORE_B = 2
N_CORES = 8
K7 = 7
PADL, PADR = 7, 3
LW = PADL + W + PADR      # padded lum/sq width (1034)
SCAN_N = W + 3            # h[:, j+3] = centered 7-sum at col j


def _tiles():
    specs = []
    specs.append(dict(r0=0, nr=128, K=128, M=125, out_r0=0, w=0))
    for t in range(1, 8):
        specs.append(dict(r0=122 * t, nr=128, K=128, M=122, out_r0=122 * t + 3, w=1))
    specs.append(dict(r0=976, nr=48, K=48, M=45, out_r0=979, w=2))
    assert specs[-1]["out_r0"] + specs[-1]["M"] == H
    return specs


def band_weights() -> np.ndarray:
    """Four [128,128] blocks: W0 | Wmid | Wlast | -441*I."""
    wb = np.zeros((128, 4 * 128), np.float32)
    for m in range(125):
        for k in range(max(m - 3, 0), m + 4):
            wb[k, m] = 1.0
    for m in range(122):
        for k in range(m, m + 7):
            wb[k, 128 + m] = 1.0
    for m in range(45):
        for k in range(m, min(m + 7, 48)):
            wb[k, 256 + m] = 1.0
    for m in range(128):
        wb[m, 384 + m] = -441.0
    return wb


def build_nc(finalize: bool = True) -> bass.Bass:
    nc = bacc.Bacc("TRN2", target_bir_lowering=False)
    f32 = mybir.dt.float32
    f32r = mybir.dt.float32r

    x = nc.dram_tensor("x", [PER_CORE_B, C, H, W], f32, kind="ExternalInput")
    wbt = nc.dram_tensor("wb", [128, 4 * 128], f32r, kind="ExternalInput")
    y = nc.dram_tensor("y", [PER_CORE_B, 1, H, W], f32, kind="ExternalOutput")

    inv147 = float(np.float32(1.0) / np.float32(147.0))
    inv441 = float(np.float32(1.0) / np.float32(441.0))

    with tile.TileContext(nc) as tc, ExitStack() as ctx:
        cpool = ctx.enter_context(tc.tile_pool(name="const", bufs=1))
        lpool = ctx.enter_context(tc.tile_pool(name="lum", bufs=4))
        xpool = ctx.enter_context(tc.tile_pool(name="x2", bufs=3))
        spool = ctx.enter_context(tc.tile_pool(name="sq", bufs=3))
        hpool = ctx.enter_context(tc.tile_pool(name="hsum", bufs=3))
        mpool = ctx.enter_context(tc.tile_pool(name="m2", bufs=3))
        vpool = ctx.enter_context(tc.tile_pool(name="vout", bufs=3))
        p1pool = ctx.enter_context(tc.tile_pool(name="ps1", bufs=2, space="PSUM"))
        p2pool = ctx.enter_context(tc.tile_pool(name="ps2", bufs=2, space="PSUM"))

        WB = cpool.tile([128, 4 * 128], f32r)
        nc.sync.dma_start(out=WB[:], in_=wbt[:, :])
        WBf = WB[:].bitcast(f32)

        for b in range(PER_CORE_B):
            for sp in _tiles():
                r0, nr, K, M, out_r0, wsel = (
                    sp["r0"], sp["nr"], sp["K"], sp["M"], sp["out_r0"], sp["w"],
                )
                lum = lpool.tile([128, LW], f32, tag="lum")
                nc.gpsimd.memset(lum[0:nr, 0:PADL], 0.0)
                nc.gpsimd.memset(lum[0:nr, PADL + W : LW], 0.0)
                # lum = x0 (HWDGE load) + x1 (SWDGE accumulate-DMA) + x2 (GPSIMD add)
                nc.sync.dma_start(
                    out=lum[0:nr, PADL : PADL + W], in_=x[b, 0, r0 : r0 + nr, :]
                )
                nc.gpsimd.dma_start(
                    out=lum[0:nr, PADL : PADL + W],
                    in_=x[b, 1, r0 : r0 + nr, :],
                    accum_op=mybir.AluOpType.add,
                )
                X2 = xpool.tile([128, W], f32, tag="X2")
                nc.sync.dma_start(out=X2[0:nr, :], in_=x[b, 2, r0 : r0 + nr, :])
                nc.gpsimd.tensor_add(
                    lum[0:nr, PADL : PADL + W], lum[0:nr, PADL : PADL + W], X2[0:nr, :]
                )

                sq = spool.tile([128, LW], f32, tag="sq")
                nc.gpsimd.memset(sq[0:nr, 0:PADL], 0.0)
                nc.gpsimd.memset(sq[0:nr, PADL + W : LW], 0.0)
                nc.scalar.activation(
                    sq[0:nr, PADL : PADL + W],
                    lum[0:nr, PADL : PADL + W],
                    mybir.ActivationFunctionType.Square,
                )

                h1 = hpool.tile([128, SCAN_N], f32r, tag="h1")
                h2 = hpool.tile([128, SCAN_N], f32, tag="h2")
                nc.vector.tensor_tensor_scan(
                    out=h1[0:nr, :],
                    data0=lum[0:nr, PADL : PADL + SCAN_N],
                    data1=lum[0:nr, 0:SCAN_N],
                    initial=0.0,
                    op0=mybir.AluOpType.add,
                    op1=mybir.AluOpType.subtract,
                )
                nc.vector.tensor_tensor_scan(
                    out=h2[0:nr, :],
                    data0=sq[0:nr, PADL : PADL + SCAN_N],
                    data1=sq[0:nr, 0:SCAN_N],
                    initial=0.0,
                    op0=mybir.AluOpType.add,
                    op1=mybir.AluOpType.subtract,
                )

                S1 = p1pool.tile([128, W], f32, tag="S1")
                S2 = p2pool.tile([128, W], f32, tag="S2")
                # mean path (float32r full-rate)
                for cnk in range(2):
                    nc.tensor.matmul(
                        S1[0:M, 512 * cnk : 512 * (cnk + 1)],
                        WB[0:K, 128 * wsel : 128 * wsel + M],
                        h1[0:K, 3 + 512 * cnk : 3 + 512 * (cnk + 1)],
                        start=True,
                        stop=True,
                    )

                m2 = mpool.tile([128, W], f32r, tag="m2")
                nc.scalar.activation(
                    m2[0:M, :],
                    S1[0:M, :],
                    mybir.ActivationFunctionType.Square,
                    scale=inv147,
                )

                # sq-path PSUM chunks: write -441*m2 first (small values, f32r
                # write rounding is harmless), then accumulate the exact fp32
                # band sum of h2 on top in fp32 PSUM.
                for cnk in range(2):
                    nc.tensor.matmul(
                        S2[0:M, 512 * cnk : 512 * (cnk + 1)],
                        WB[0:M, 384 : 384 + M],
                        m2[0:M, 512 * cnk : 512 * (cnk + 1)],
                        start=True,
                        stop=False,
                    )
                for cnk in range(2):
                    nc.tensor.matmul(
                        S2[0:M, 512 * cnk : 512 * (cnk + 1)],
                        WBf[0:K, 128 * wsel : 128 * wsel + M],
                        h2[0:K, 3 + 512 * cnk : 3 + 512 * (cnk + 1)],
                        start=False,
                        stop=True,
                    )

                V = vpool.tile([128, W], f32, tag="V")
                nc.scalar.activation(
                    V[0:M, :],
                    S2[0:M, :],
                    mybir.ActivationFunctionType.Copy,
                    scale=inv441,
                )

                # SWDGE out: spreads across all 16 SDMA engines
                nc.gpsimd.dma_start(
                    out=y[b, 0, out_r0 : out_r0 + M, :], in_=V[0:M, :]
                )

    if finalize:
        nc.finalize()
    return nc


def kernel(x, kernel_size):
    assert int(kernel_size) == K7
    x = np.ascontiguousarray(np.asarray(x, dtype=np.float32))
    B = x.shape[0]
    assert x.shape == (B, C, H, W) and B == PER_CORE_B * N_CORES

    from concourse.bass_utils import run_bass_kernel_spmd

    nc = build_nc()
    wb = band_weights()
    in_maps = [
        {"x": x[i * PER_CORE_B : (i + 1) * PER_CORE_B], "wb": wb}
        for i in range(N_CORES)
    ]
    res = run_bass_kernel_spmd(nc, in_maps, list(range(N_CORES)))
    y = np.concatenate([res.results[i]["y"] for i in range(N_CORES)], axis=0)
    return y
